# revision 43
# baseline (speedup 1.0000x reference)
"""GNN message-passing ConvNet layer on 8 TRN2 NeuronCores (Bass/Tile).

Computes, for x [B=4, N=4096, D=128], adj_mat [B, N, N] (0/1 floats),
U [D, D]:
    deg[b, i] = sum_j adj_mat[b, j, i]
    agg[b, i, :] = sum_j adj[b, j, i] * x[b, j, :]
    out = relu((agg @ U) / deg[..., None])

Sharding: core c handles batch c//2 and destination-node half c%2 — no
collectives; each core reads its own adjacency column slice once.

Kernel design (per core, memory-bound; ~70 us vs 122 us baseline):
  - Associativity: (A^T x) U == A^T (x U). Precompute y = x @ U once on
    the PE (bf16), then the adjacency pass emits the final pre-relu
    output directly (no U-matmul tail).
  - adj is 0/1: cast to fp8e4 on host (exact) -> 8.4 MiB HBM traffic
    per core (4x less than fp32). Host-packs to the SBUF tile stream
    order [p, v, q, k, n] so every DMA line is contiguous per
    partition; the k axis pairs j-tiles (j, j+128) for DoubleRow.
  - agg matmuls mix dtypes: bf16 y stationary x fp8 adj moving (legal:
    only fp32 operands must match dtypes).
  - deg runs on the PE in DoubleRow fp8 mode (pair-of-ones stationary
    [128,2,16] - dim 16 keeps the LDWEIGHTS pair-axis step%16==0 - and
    the k-paired adjacency moving): 256-row contraction in 512 cycles,
    half the cost of the normal-mode equivalent.
  - j-outer loop: stationary y_t serves all 4 i-rounds (amortizes
    LDWEIGHTS, which profiling showed serializes at ~45ns/MM);
    input DMAs issue in priority order (xT before adjacency bulk) so
    the y-precompute isn't stuck behind fair-shared rings.
  - Last chunk runs deg first and orders agg matmuls q-outer so each
    round's recip/broadcast/relu/scale/store overlaps remaining MMs.
"""

import os
import sys

for _p in ("/opt/trn_rl_repo",):
    if _p not in sys.path and os.path.isdir(_p):
        sys.path.insert(0, _p)

from contextlib import ExitStack

import numpy as np

B, N, D = 4, 4096, 128
P = 128
N_CORES = 8

_PROG = None


def _build_program(n=N, i_core=N // 2, d=D, w=512, jt_per_dma=4):
    from concourse import mybir, tile, bacc

    f32 = mybir.dt.float32
    f16 = mybir.dt.float16
    bf16 = mybir.dt.bfloat16
    fp8 = mybir.dt.float8e4
    n_jt = n // P              # 32 j-tiles of 128
    n_rounds = i_core // w     # 4 i-rounds of 512
    n_chunks = n_jt // jt_per_dma
    xt_chunk = 8

    nc = bacc.Bacc(
        "TRN2",
        target_bir_lowering=False,
        debug=False,
        enable_asserts=True,
        num_devices=N_CORES,
    )
    # host-packed: adj_p[p, v, q, k, n] = adj[b, v*256+k*128+p, i0+q*512+n]
    n_v = n_jt // 2
    adj_d = nc.dram_tensor(
        "adj_p", [P, n_v, n_rounds, 2, w], fp8, kind="ExternalInput"
    )
    # host-packed transpose: xT_p[d, t, j] = x[b, t*128+j, d]
    xt_d = nc.dram_tensor("xT_p", [P, n_jt, d], bf16, kind="ExternalInput")
    u_d = nc.dram_tensor("U", [d, d], bf16, kind="ExternalInput")
    ones_d = nc.dram_tensor("ones_c", [P, 1], bf16, kind="ExternalInput")
    # out_sp[q, e, n] = out[b, i0 + q*512 + n, e]  (host transposes back)
    out_d = nc.dram_tensor("out_sp", [n_rounds, d, w], bf16, kind="ExternalOutput")

    with tile.TileContext(nc, trace_sim=False) as tc, ExitStack() as ctx:
        const_pool = ctx.enter_context(tc.tile_pool(name="const", bufs=1))
        y_pool = ctx.enter_context(tc.tile_pool(name="y", bufs=1))
        adj_pool = ctx.enter_context(tc.tile_pool(name="adj", bufs=5))
        scale_pool = ctx.enter_context(tc.tile_pool(name="scale", bufs=4))
        out_pool = ctx.enter_context(tc.tile_pool(name="out", bufs=8))
        small_pool = ctx.enter_context(tc.tile_pool(name="small", bufs=4))
        ps_agg = ctx.enter_context(tc.tile_pool(name="ps_agg", bufs=1, space="PSUM"))
        # 4 banks shared in time: y-precompute tiles first, then one deg
        # accumulator per i-round.
        ps_aux = ctx.enter_context(tc.tile_pool(name="ps_aux", bufs=1, space="PSUM"))

        u_sb = const_pool.tile([P, d], bf16)
        nc.scalar.dma_start(u_sb[:], u_d[:])
        # fp8 pair-of-ones stationary for the DoubleRow deg matmuls
        # [128, 2, 16] pair-of-ones: dim width 16 keeps the DoubleRow
        # LDWEIGHTS pair-axis step a multiple of 16 bytes (ISA rule).
        ones_f8 = const_pool.tile([P, 2, 16], fp8)
        nc.vector.memset(ones_f8[:], 1.0)
        ones = const_pool.tile([P, 1], bf16)
        nc.scalar.dma_start(ones[:], ones_d[:])
        xt_sb = const_pool.tile([P, n_jt, d], bf16)
        for xc in range(n_jt // xt_chunk):
            nc.scalar.dma_start(
                xt_sb[:, xc * xt_chunk : (xc + 1) * xt_chunk, :],
                xt_d[:, xc * xt_chunk : (xc + 1) * xt_chunk, :],
            )

        # Phase 0: y = x @ U (bf16), laid out [j_in_tile, t, e].
        y_sb = y_pool.tile([P, n_jt, d], bf16)
        for t in range(n_jt):
            y_ps = ps_aux.tile([P, d], f32, tag=f"d{t % 4}", name=f"y{t}")
            nc.tensor.matmul(y_ps[:], xt_sb[:, t, :], u_sb[:], start=True, stop=True)
            nc.scalar.activation(
                y_sb[:, t, :], y_ps[:], mybir.ActivationFunctionType.Copy
            )

        # Phase 1: stream adjacency once. agg via normal mixed-dtype
        # matmuls (bf16 y x fp8 adj); deg via DoubleRow fp8 matmuls
        # (pair-of-ones stationary, k-paired adjacency moving: 256-row
        # contraction in 512 cycles).
        agg_ps = [ps_agg.tile([P, w], f32, tag=f"agg{q}", name=f"agg{q}")
                  for q in range(n_rounds)]
        deg_ps = [ps_aux.tile([16, w], f32, tag=f"d{q}", name=f"deg{q}")
                  for q in range(n_rounds)]
        v_per_dma = jt_per_dma // 2
        for c in range(n_chunks):
            adj_sb = adj_pool.tile(
                [P, v_per_dma, n_rounds, 2, w], fp8, tag="adj"
            )
            nc.sync.dma_start(
                adj_sb[:], adj_d[:, c * v_per_dma : (c + 1) * v_per_dma, :, :, :]
            )
            first, last = c == 0, c == n_chunks - 1

            def deg_mms():
                for vi in range(v_per_dma):
                    for q in range(n_rounds):
                        nc.tensor.matmul(
                            deg_ps[q][:],
                            ones_f8[:],
                            adj_sb[:, vi, q, :, :],
                            start=(first and vi == 0),
                            stop=(last and vi == v_per_dma - 1),
                            perf_mode=mybir.MatmulPerfMode.DoubleRow,
                        )

            def agg_mms():
                if last:
                    order = [(vi, k, q) for q in range(n_rounds)
                             for vi in range(v_per_dma) for k in range(2)]
                else:
                    order = [(vi, k, q) for vi in range(v_per_dma)
                             for k in range(2) for q in range(n_rounds)]
                for vi, k, q in order:
                    t = 2 * (c * v_per_dma + vi) + k
                    nc.tensor.matmul(
                        agg_ps[q][:],
                        y_sb[:, t, :],
                        adj_sb[:, vi, q, k, :],
                        start=(first and vi == 0 and k == 0),
                        stop=(last and vi == v_per_dma - 1 and k == 1),
                    )

            if last:
                deg_mms()
                agg_mms()
            else:
                agg_mms()
                deg_mms()

        for q in range(n_rounds):
            recip = small_pool.tile([1, w], f32, tag="recip")
            nc.vector.reciprocal_approx_fast(recip[:], deg_ps[q][0:1, :])
            rb = scale_pool.tile([P, w], f32, tag="rb")
            nc.gpsimd.partition_broadcast(rb[:], recip[:])
            relu_sb = out_pool.tile([P, w], f32, tag="relu")
            nc.scalar.activation(
                relu_sb[:], agg_ps[q][:], mybir.ActivationFunctionType.Relu
            )
            out_sb = out_pool.tile([P, w], bf16, tag="osb")
            nc.vector.tensor_mul(out_sb[:], relu_sb[:], rb[:])
            (nc.scalar if q % 2 else nc.gpsimd).dma_start(out_d[q, :, :], out_sb[:])

    nc.compile()
    return nc


def _get_program():
    global _PROG
    if _PROG is None:
        _PROG = _build_program()
    return _PROG


def _shard_inputs(x, adj_mat, U):
    import ml_dtypes

    bf16 = ml_dtypes.bfloat16
    fp8 = ml_dtypes.float8_e4m3
    i_core = N // 2
    n_jt = N // P
    n_rounds = i_core // 512
    ones_c = np.ones((P, 1), dtype=bf16)
    u_f = np.ascontiguousarray(U.astype(bf16))
    adj_f8 = adj_mat.astype(fp8)  # exact: values are 0/1
    in_maps = []
    for c in range(N_CORES):
        b, half = c // 2, c % 2
        i0 = half * i_core
        # [N, i_core] -> [v, k, p, q, n] -> [p, v, q, k, n]
        a = adj_f8[b, :, i0 : i0 + i_core].reshape(n_jt // 2, 2, P, n_rounds, 512)
        a = np.ascontiguousarray(a.transpose(2, 0, 3, 1, 4))
        xt = np.ascontiguousarray(
            x[b].reshape(n_jt, P, D).transpose(2, 0, 1)
        ).astype(bf16)
        in_maps.append(
            {"adj_p": a, "xT_p": xt, "U": u_f, "ones_c": ones_c}
        )
    return in_maps


def _run(x, adj_mat, U, trace=False):
    from concourse.bass_utils import run_bass_kernel_spmd

    nc = _get_program()
    in_maps = _shard_inputs(x, adj_mat, U)
    res = run_bass_kernel_spmd(
        nc, in_maps, core_ids=list(range(N_CORES)), trace=trace
    )
    i_core = N // 2
    out = np.empty((B, N, D), dtype=np.float32)
    for c in range(N_CORES):
        b, half = c // 2, c % 2
        i0 = half * i_core
        osp = res.results[c]["out_sp"].astype(np.float32)  # [q, e, n]
        out[b, i0 : i0 + i_core, :] = osp.transpose(0, 2, 1).reshape(i_core, D)
    return out, res


def kernel(x, adj_mat, U):
    out, _ = _run(
        np.asarray(x, dtype=np.float32),
        np.asarray(adj_mat, dtype=np.float32),
        np.asarray(U, dtype=np.float32),
    )
    return out


# revision 44
# speedup vs baseline: 1.0073x; 1.0073x over previous
"""GNN message-passing ConvNet layer on 8 TRN2 NeuronCores (Bass/Tile).

Computes, for x [B=4, N=4096, D=128], adj_mat [B, N, N] (0/1 floats),
U [D, D]:
    deg[b, i] = sum_j adj_mat[b, j, i]
    agg[b, i, :] = sum_j adj[b, j, i] * x[b, j, :]
    out = relu((agg @ U) / deg[..., None])

Sharding: core c handles batch c//2 and destination-node half c%2 — no
collectives; each core reads its own adjacency column slice once.

Kernel design (per core, memory-bound; ~70 us vs 122 us baseline):
  - Associativity: (A^T x) U == A^T (x U). Precompute y = x @ U once on
    the PE (bf16), then the adjacency pass emits the final pre-relu
    output directly (no U-matmul tail).
  - adj is 0/1: cast to fp8e4 on host (exact) -> 8.4 MiB HBM traffic
    per core (4x less than fp32). Host-packs to the SBUF tile stream
    order [p, v, q, k, n] so every DMA line is contiguous per
    partition; the k axis pairs j-tiles (j, j+128) for DoubleRow.
  - agg matmuls mix dtypes: bf16 y stationary x fp8 adj moving (legal:
    only fp32 operands must match dtypes).
  - deg runs on the PE in DoubleRow fp8 mode (pair-of-ones stationary
    [128,2,16] - dim 16 keeps the LDWEIGHTS pair-axis step%16==0 - and
    the k-paired adjacency moving): 256-row contraction in 512 cycles,
    half the cost of the normal-mode equivalent.
  - j-outer loop: stationary y_t serves all 4 i-rounds (amortizes
    LDWEIGHTS, which profiling showed serializes at ~45ns/MM);
    input DMAs issue in priority order (xT before adjacency bulk) so
    the y-precompute isn't stuck behind fair-shared rings.
  - Last chunk runs deg first and orders agg matmuls q-outer so each
    round's recip/broadcast/relu/scale/store overlaps remaining MMs.
"""

import os
import sys

for _p in ("/opt/trn_rl_repo",):
    if _p not in sys.path and os.path.isdir(_p):
        sys.path.insert(0, _p)

from contextlib import ExitStack

import numpy as np

B, N, D = 4, 4096, 128
P = 128
N_CORES = 8

_PROG = None


def _build_program(n=N, i_core=N // 2, d=D, w=512, jt_per_dma=4):
    from concourse import mybir, tile, bacc

    f32 = mybir.dt.float32
    f16 = mybir.dt.float16
    bf16 = mybir.dt.bfloat16
    fp8 = mybir.dt.float8e4
    n_jt = n // P              # 32 j-tiles of 128
    n_rounds = i_core // w     # 4 i-rounds of 512
    n_chunks = n_jt // jt_per_dma
    xt_chunk = 8

    nc = bacc.Bacc(
        "TRN2",
        target_bir_lowering=False,
        debug=False,
        enable_asserts=True,
        num_devices=N_CORES,
    )
    # host-packed: adj_p[p, v, q, k, n] = adj[b, v*256+k*128+p, i0+q*512+n]
    n_v = n_jt // 2
    adj_d = nc.dram_tensor(
        "adj_p", [P, n_v, n_rounds, 2, w], fp8, kind="ExternalInput"
    )
    # host-packed transpose: xT_p[d, t, j] = x[b, t*128+j, d]
    xt_d = nc.dram_tensor("xT_p", [P, n_jt, d], bf16, kind="ExternalInput")
    u_d = nc.dram_tensor("U", [d, d], bf16, kind="ExternalInput")
    ones_d = nc.dram_tensor("ones_c", [P, 1], bf16, kind="ExternalInput")
    # out_sp[q, e, n] = out[b, i0 + q*512 + n, e]  (host transposes back)
    out_d = nc.dram_tensor("out_sp", [n_rounds, d, w], bf16, kind="ExternalOutput")

    with tile.TileContext(nc, trace_sim=False) as tc, ExitStack() as ctx:
        const_pool = ctx.enter_context(tc.tile_pool(name="const", bufs=1))
        y_pool = ctx.enter_context(tc.tile_pool(name="y", bufs=1))
        adj_pool = ctx.enter_context(tc.tile_pool(name="adj", bufs=5))
        scale_pool = ctx.enter_context(tc.tile_pool(name="scale", bufs=4))
        out_pool = ctx.enter_context(tc.tile_pool(name="out", bufs=8))
        small_pool = ctx.enter_context(tc.tile_pool(name="small", bufs=4))
        ps_agg = ctx.enter_context(tc.tile_pool(name="ps_agg", bufs=1, space="PSUM"))
        # 4 banks shared in time: y-precompute tiles first, then one deg
        # accumulator per i-round.
        ps_aux = ctx.enter_context(tc.tile_pool(name="ps_aux", bufs=1, space="PSUM"))

        u_sb = const_pool.tile([P, d], bf16)
        nc.scalar.dma_start(u_sb[:], u_d[:])
        # fp8 pair-of-ones stationary for the DoubleRow deg matmuls
        # [128, 2, 16] pair-of-ones: dim width 16 keeps the DoubleRow
        # LDWEIGHTS pair-axis step a multiple of 16 bytes (ISA rule).
        ones_f8 = const_pool.tile([P, 2, 16], fp8)
        nc.vector.memset(ones_f8[:], 1.0)
        ones = const_pool.tile([P, 1], bf16)
        nc.scalar.dma_start(ones[:], ones_d[:])
        xt_sb = const_pool.tile([P, n_jt, d], bf16)
        for xc in range(n_jt // xt_chunk):
            nc.sync.dma_start(
                xt_sb[:, xc * xt_chunk : (xc + 1) * xt_chunk, :],
                xt_d[:, xc * xt_chunk : (xc + 1) * xt_chunk, :],
            )

        # Phase 0: y = x @ U (bf16), laid out [j_in_tile, t, e].
        y_sb = y_pool.tile([P, n_jt, d], bf16)
        for t in range(n_jt):
            y_ps = ps_aux.tile([P, d], f32, tag=f"d{t % 4}", name=f"y{t}")
            nc.tensor.matmul(y_ps[:], xt_sb[:, t, :], u_sb[:], start=True, stop=True)
            nc.scalar.activation(
                y_sb[:, t, :], y_ps[:], mybir.ActivationFunctionType.Copy
            )

        # Phase 1: stream adjacency once. agg via normal mixed-dtype
        # matmuls (bf16 y x fp8 adj); deg via DoubleRow fp8 matmuls
        # (pair-of-ones stationary, k-paired adjacency moving: 256-row
        # contraction in 512 cycles).
        agg_ps = [ps_agg.tile([P, w], f32, tag=f"agg{q}", name=f"agg{q}")
                  for q in range(n_rounds)]
        deg_ps = [ps_aux.tile([16, w], f32, tag=f"d{q}", name=f"deg{q}")
                  for q in range(n_rounds)]
        v_per_dma = jt_per_dma // 2
        for c in range(n_chunks):
            adj_sb = adj_pool.tile(
                [P, v_per_dma, n_rounds, 2, w], fp8, tag="adj"
            )
            nc.sync.dma_start(
                adj_sb[:], adj_d[:, c * v_per_dma : (c + 1) * v_per_dma, :, :, :]
            )
            first, last = c == 0, c == n_chunks - 1

            def deg_mms():
                for vi in range(v_per_dma):
                    for q in range(n_rounds):
                        nc.tensor.matmul(
                            deg_ps[q][:],
                            ones_f8[:],
                            adj_sb[:, vi, q, :, :],
                            start=(first and vi == 0),
                            stop=(last and vi == v_per_dma - 1),
                            perf_mode=mybir.MatmulPerfMode.DoubleRow,
                        )

            def agg_mms():
                if last:
                    order = [(vi, k, q) for q in range(n_rounds)
                             for vi in range(v_per_dma) for k in range(2)]
                else:
                    order = [(vi, k, q) for vi in range(v_per_dma)
                             for k in range(2) for q in range(n_rounds)]
                for vi, k, q in order:
                    t = 2 * (c * v_per_dma + vi) + k
                    nc.tensor.matmul(
                        agg_ps[q][:],
                        y_sb[:, t, :],
                        adj_sb[:, vi, q, k, :],
                        start=(first and vi == 0 and k == 0),
                        stop=(last and vi == v_per_dma - 1 and k == 1),
                    )

            if last:
                deg_mms()
                agg_mms()
            else:
                agg_mms()
                deg_mms()

        for q in range(n_rounds):
            recip = small_pool.tile([1, w], f32, tag="recip")
            nc.vector.reciprocal_approx_fast(recip[:], deg_ps[q][0:1, :])
            rb = scale_pool.tile([P, w], f32, tag="rb")
            nc.gpsimd.partition_broadcast(rb[:], recip[:])
            relu_sb = out_pool.tile([P, w], f32, tag="relu")
            nc.scalar.activation(
                relu_sb[:], agg_ps[q][:], mybir.ActivationFunctionType.Relu
            )
            out_sb = out_pool.tile([P, w], bf16, tag="osb")
            nc.vector.tensor_mul(out_sb[:], relu_sb[:], rb[:])
            (nc.scalar if q % 2 else nc.gpsimd).dma_start(out_d[q, :, :], out_sb[:])

    nc.compile()
    return nc


def _get_program():
    global _PROG
    if _PROG is None:
        _PROG = _build_program()
    return _PROG


def _shard_inputs(x, adj_mat, U):
    import ml_dtypes

    bf16 = ml_dtypes.bfloat16
    fp8 = ml_dtypes.float8_e4m3
    i_core = N // 2
    n_jt = N // P
    n_rounds = i_core // 512
    ones_c = np.ones((P, 1), dtype=bf16)
    u_f = np.ascontiguousarray(U.astype(bf16))
    adj_f8 = adj_mat.astype(fp8)  # exact: values are 0/1
    in_maps = []
    for c in range(N_CORES):
        b, half = c // 2, c % 2
        i0 = half * i_core
        # [N, i_core] -> [v, k, p, q, n] -> [p, v, q, k, n]
        a = adj_f8[b, :, i0 : i0 + i_core].reshape(n_jt // 2, 2, P, n_rounds, 512)
        a = np.ascontiguousarray(a.transpose(2, 0, 3, 1, 4))
        xt = np.ascontiguousarray(
            x[b].reshape(n_jt, P, D).transpose(2, 0, 1)
        ).astype(bf16)
        in_maps.append(
            {"adj_p": a, "xT_p": xt, "U": u_f, "ones_c": ones_c}
        )
    return in_maps


def _run(x, adj_mat, U, trace=False):
    from concourse.bass_utils import run_bass_kernel_spmd

    nc = _get_program()
    in_maps = _shard_inputs(x, adj_mat, U)
    res = run_bass_kernel_spmd(
        nc, in_maps, core_ids=list(range(N_CORES)), trace=trace
    )
    i_core = N // 2
    out = np.empty((B, N, D), dtype=np.float32)
    for c in range(N_CORES):
        b, half = c // 2, c % 2
        i0 = half * i_core
        osp = res.results[c]["out_sp"].astype(np.float32)  # [q, e, n]
        out[b, i0 : i0 + i_core, :] = osp.transpose(0, 2, 1).reshape(i_core, D)
    return out, res


def kernel(x, adj_mat, U):
    out, _ = _run(
        np.asarray(x, dtype=np.float32),
        np.asarray(adj_mat, dtype=np.float32),
        np.asarray(U, dtype=np.float32),
    )
    return out
